# revision 15
# baseline (speedup 1.0000x reference)
"""Trainium2 Bass kernel for nn_AttentionHead (B=2, S=2048, D=768, H=12).

Sharding: 8 cores = 2 batches x 4 head-groups (3 heads each).
Per core: QKV projection for its heads (transposed layout), causal
attention with softmax over the QUERY axis (reference peculiarity:
softmax dim=-2, scaled by sqrt(d_model)), AllGather of per-head outputs
within each batch's 4-core group, then a column-slice of the output
projection.  Host only slices / transposes / casts / concatenates.

v3 schedule notes:
  - All large inputs are pre-cast to bf16 on the host and DMA'd directly
    into matmul-ready SBUF tiles (no on-chip casts; half the HBM bytes).
  - Scores are built transposed (S_T[k, q]) so the softmax axis (q) is
    the SBUF free axis; ScalarE exp computes per-k row sums via
    accum_out.  The per-k normalizer is folded into V ("V'") so the
    attn @ v matmul consumes raw exp scores.
  - The q-projection bias is dropped: a per-k-row constant shifts all
    logits in the softmax (query) axis equally and cancels exactly.
  - One unified wave processes all three heads per ki block, keeping the
    PE and the (bottleneck) Scalar exp stream balanced end to end.
  - Lazy AV: for ki<8 the attn@v product is applied only to columns
    <1024; the exp tiles for columns >=1024 are retained in SBUF and
    applied at steps 8..11.  This keeps only 3 PSUM banks of AV state
    live, freeing 4 banks for two [128,1024] score arenas (fewer, wider
    exp instructions).
  - Four combined [192,512] AllGathers (one per 512-column block) carry
    all three heads; a dummy AllGather at t=0 absorbs the collective
    cold-start / barrier.  The output projection is interleaved into the
    wave as AllGather results land.
"""

import math

import numpy as np

B, S, D, H, DH = 2, 2048, 768, 12, 64
NCORES = 8
GROUPS = 4  # head-groups per batch
HPG = 3  # heads per group
EPG = HPG * DH  # 192
SCALE = 1.0 / math.sqrt(D)
NEG = -1.0e30

_cache = {}


def _build():
    import contextlib

    import concourse.bacc as bacc
    import concourse.mybir as mybir
    from concourse import tile

    f32 = mybir.dt.float32
    bf16 = mybir.dt.bfloat16
    EXP = mybir.ActivationFunctionType.Exp

    nc = bacc.Bacc("TRN2", target_bir_lowering=False, debug=False, num_devices=NCORES)

    xT = nc.dram_tensor("xT", [D, S], bf16, kind="ExternalInput")
    wqk = nc.dram_tensor("wqk", [D, 384], bf16, kind="ExternalInput")
    wv = nc.dram_tensor("wv", [D, EPG], bf16, kind="ExternalInput")
    wout = nc.dram_tensor("wout", [D, EPG], bf16, kind="ExternalInput")
    bqkc = nc.dram_tensor("bqkc", [128, 2], f32, kind="ExternalInput")
    bvv = nc.dram_tensor("bvv", [1, EPG], bf16, kind="ExternalInput")
    boutc = nc.dram_tensor("boutc", [128, 2], f32, kind="ExternalInput")
    tri = nc.dram_tensor("tri", [128, 128], f32, kind="ExternalInput")
    out = nc.dram_tensor("out", [EPG, S], f32, kind="ExternalOutput")

    ag_w = [512, 512, 512, 256, 256]
    ag_in = [
        nc.dram_tensor(f"ag_in{f}", [EPG, ag_w[f]], bf16) for f in range(5)
    ]
    ag_out = [
        nc.dram_tensor(f"ag_out{f}", [4 * EPG, ag_w[f]], bf16) for f in range(5)
    ]
    ag_ind = nc.dram_tensor("ag_ind", [64, 16], bf16)
    ag_outd = nc.dram_tensor("ag_outd", [256, 16], bf16)

    groups = [[0, 1, 2, 3], [4, 5, 6, 7]]

    with tile.TileContext(nc) as tc:
        with contextlib.ExitStack() as ctx:
            const_p = ctx.enter_context(tc.tile_pool(name="const", bufs=1))
            w_p = ctx.enter_context(tc.tile_pool(name="w", bufs=1))
            xt_p = ctx.enter_context(tc.tile_pool(name="xt", bufs=1))
            qk_p = ctx.enter_context(tc.tile_pool(name="qk", bufs=1))
            v_p = ctx.enter_context(tc.tile_pool(name="v", bufs=1))
            e_p = ctx.enter_context(tc.tile_pool(name="e", bufs=10))
            ed_p = ctx.enter_context(tc.tile_pool(name="ed", bufs=26))
            st_p = ctx.enter_context(tc.tile_pool(name="stat", bufs=24))
            vp_p = ctx.enter_context(tc.tile_pool(name="vp", bufs=30))
            atn_p = ctx.enter_context(tc.tile_pool(name="atn", bufs=1))
            ag_p = ctx.enter_context(tc.tile_pool(name="ag", bufs=1))
            o_p = ctx.enter_context(tc.tile_pool(name="o", bufs=2))
            arena = ctx.enter_context(tc.tile_pool(name="arena", bufs=2, space="PSUM"))
            vps = ctx.enter_context(tc.tile_pool(name="vps", bufs=1, space="PSUM"))
            avp = ctx.enter_context(tc.tile_pool(name="avp", bufs=1, space="PSUM"))

            # ---- constants (tiny DMAs first) ----
            bqk_c = const_p.tile([128, 2], f32)
            nc.sync.dma_start(bqk_c[:], bqkc[:, :])
            bout_c = const_p.tile([128, 2], f32)
            nc.sync.dma_start(bout_c[:], boutc[:, :])
            tri_f = const_p.tile([128, 128], f32)
            nc.sync.dma_start(tri_f[:], tri[:, :])
            bv_t = const_p.tile([1, EPG], bf16)
            nc.sync.dma_start(bv_t[:], bvv[:, :])
            ones = const_p.tile([1, 512], bf16)
            nc.vector.memset(ones[:], 1.0)
            warm = const_p.tile([128, 512], bf16)
            nc.vector.memset(warm[:], 0.0)
            dmy = const_p.tile([64, 16], bf16)
            nc.vector.memset(dmy[:], 0.0)

            # dummy AllGather: absorb the collective bootstrap barrier and
            # ncfw cold start while the PE is busy with QKV
            nc.sync.dma_start(ag_ind[:, :], dmy[:])
            nc.gpsimd.collective_compute(
                "AllGather",
                mybir.AluOpType.bypass,
                replica_groups=groups,
                ins=[ag_ind.ap().opt()],
                outs=[ag_outd.ap().opt()],
            )

            # ---- input DMAs: wqk first, then x halves, then wv, wout ----
            wqk_t = []
            for dt_i in range(6):
                wt = w_p.tile([128, 384], bf16, tag=f"wqk{dt_i}")
                nc.sync.dma_start(wt[:], wqk[dt_i * 128 : (dt_i + 1) * 128, :])
                wqk_t.append(wt)
            xt_t = []
            for dt_i in range(6):
                xt = xt_p.tile([128, S], bf16, tag=f"xt{dt_i}")
                nc.sync.dma_start(
                    xt[:, 0:1024], xT[dt_i * 128 : (dt_i + 1) * 128, 0:1024]
                )
                xt_t.append(xt)
            for dt_i in range(6):
                nc.sync.dma_start(
                    xt_t[dt_i][:, 1024:2048],
                    xT[dt_i * 128 : (dt_i + 1) * 128, 1024:2048],
                )
            wv_t, wout_t = [], []
            for dt_i in range(6):
                vt = w_p.tile([128, EPG], bf16, tag=f"wv{dt_i}")
                nc.sync.dma_start(vt[:], wv[dt_i * 128 : (dt_i + 1) * 128, :])
                wv_t.append(vt)
            for dt_i in range(6):
                wo = w_p.tile([128, EPG], bf16, tag=f"wout{dt_i}")
                nc.sync.dma_start(wo[:], wout[dt_i * 128 : (dt_i + 1) * 128, :])
                wout_t.append(wo)

            # ---- persistent SBUF tiles ----
            k01 = qk_p.tile([128, S], bf16, tag="k01")
            q01 = qk_p.tile([128, S], bf16, tag="q01")
            qk2 = qk_p.tile([128, S], bf16, tag="qk2")
            k2 = qk_p.tile([64, S], bf16, tag="k2")
            q2d = qk_p.tile([128, S], bf16, tag="q2d")
            vnat = v_p.tile([128, 16 * EPG], bf16)
            atn01 = atn_p.tile([128, S], bf16, tag="atn01")
            atn2 = atn_p.tile([64, S], bf16, tag="atn2")
            ag_t = {}
            for fb in range(4):
                for dt_i in range(6):
                    ag_t[(fb, dt_i)] = ag_p.tile(
                        [128, 512], bf16, tag=f"ag{fb}_{dt_i}", name=f"ag{fb}_{dt_i}"
                    )

            # ---- PE warm-up while first DMAs land ----
            for _ in range(12):
                wps = arena.tile([128, 1024], f32, tag="strip")
                nc.tensor.matmul(
                    wps[:, 0:512], warm[:, 0:128], warm[:],
                    start=True, stop=True, skip_group_check=True,
                )

            # qkv chunk: column group c (128 wide), seq chunk sc (1024 wide)
            def qkv_chunk(dst, c, sc, bias_col):
                pt = arena.tile([128, 1024], f32, tag="strip")
                for off in range(0, 1024, 512):
                    for dt_i in range(6):
                        nc.tensor.matmul(
                            pt[:, off : off + 512],
                            wqk_t[dt_i][:, c * 128 : (c + 1) * 128],
                            xt_t[dt_i][:, sc * 1024 + off : sc * 1024 + off + 512],
                            start=(dt_i == 0), stop=(dt_i == 5),
                        )
                if bias_col is None:
                    nc.vector.tensor_copy(dst[:, sc * 1024 : (sc + 1) * 1024], pt[:])
                else:
                    nc.vector.tensor_scalar_add(
                        dst[:, sc * 1024 : (sc + 1) * 1024],
                        pt[:],
                        bqk_c[:, bias_col : bias_col + 1],
                    )

            def v_tile(st_i):
                pt = vps.tile([128, 512], f32, tag="vstrip")
                p = pt[:, 0:EPG]
                nc.tensor.matmul(p, ones[:, 0:128], bv_t[:], start=True, stop=False)
                for dt_i in range(6):
                    nc.tensor.matmul(
                        p,
                        xt_t[dt_i][:, st_i * 128 : (st_i + 1) * 128],
                        wv_t[dt_i][:],
                        start=False, stop=(dt_i == 5),
                    )
                nc.vector.tensor_copy(vnat[:, st_i * EPG : (st_i + 1) * EPG], p)

            # ---- AV state: 3 PSUM banks, block pairs recycled mid-wave ----
            av01_t = {}  # block -> [128,512] tile (h0 rows 0-63, h1 64-127)
            av2_t = {}  # pair index -> [128,512] tile (block 2t / 2t+1 halves)

            def av_slice(hi, b):
                if hi < 2:
                    if b not in av01_t:
                        av01_t[b] = avp.tile(
                            [128, 512], f32, tag="av01", name=f"av01_{b}", bufs=2
                        )
                    return av01_t[b][64 * hi : 64 * hi + 64, :]
                t = b // 2
                if t not in av2_t:
                    av2_t[t] = avp.tile([128, 512], f32, tag="av2", name=f"av2_{t}")
                return av2_t[t][(b % 2) * 64 : (b % 2) * 64 + 64, :]

            STOP_KI = {0: 3, 1: 7, 2: 11, 3: 15}

            def av_apply(ki, hi, vpt, et, qp0, plen, lo, hi_col, first):
                """AV chunks for global cols [lo, hi_col) of an et part."""
                q = lo
                while q < hi_col:
                    b = q // 512
                    qe = min((b + 1) * 512, hi_col)
                    dst = av_slice(hi, b)
                    nc.tensor.matmul(
                        dst[:, q - 512 * b : qe - 512 * b],
                        vpt[:],
                        et[:, q - qp0 : qe - qp0],
                        start=first, stop=(ki == STOP_KI[b]),
                        skip_group_check=True,
                    )
                    q = qe

            # score strip part: S_T[k, q] for q in [p0, p1); exp w/ accum
            def strip_part(ki, hi, p0, p1, deferred):
                kT, kb, qT, qb = srcs(ki, hi)
                plen = p1 - p0
                t = arena.tile([128, 1024], f32, tag="strip")
                off = 0
                while off < plen:
                    n = min(512, plen - off)
                    nc.tensor.matmul(
                        t[:, off : off + n],
                        kT[kb : kb + 64, ki * 128 : (ki + 1) * 128],
                        qT[qb : qb + 64, p0 + off : p0 + off + n],
                        start=True, stop=True, skip_group_check=True,
                    )
                    off += n
                if p0 == 128 * ki:
                    nc.vector.tensor_add(t[:, 0:128], t[:, 0:128], tri_f[:])
                et = (ed_p if deferred else e_p).tile(
                    [128, 1024], bf16, tag="ed" if deferred else "e"
                )
                acc = st_p.tile([128, 1], f32, tag="acc")
                nc.scalar.activation(
                    et[:, 0:plen], t[:, 0:plen], EXP, scale=SCALE, accum_out=acc[:]
                )
                return et, acc

            def srcs(ki, hi):
                if hi == 0:
                    return (k01, 0, q01, 0)
                if hi == 1:
                    return (k01, 64, q01, 64)
                return (k2, 0, qk2, 0) if ki % 2 == 0 else (qk2, 64, q2d, 64)

            def make_vpt(accs, ki, head):
                a = accs[0]
                for extra in accs[1:]:
                    s = st_p.tile([128, 1], f32, tag="ssum")
                    nc.vector.tensor_add(s[:], a[:], extra[:])
                    a = s
                rcp = st_p.tile([128, 1], f32, tag="rcp")
                nc.vector.reciprocal(rcp[:], a[:])
                vpt = vp_p.tile([128, 64], bf16, tag="vp")
                nc.vector.tensor_scalar_mul(
                    vpt[:],
                    vnat[:, ki * EPG + head * 64 : ki * EPG + (head + 1) * 64],
                    rcp[:],
                )
                return vpt

            def flush(u, c0, c1):
                w = c1 - c0
                fb = c0 // 512
                lo = c0 - 512 * fb
                cols = slice(c0, c1)
                bcols = slice(lo, lo + w)
                nc.vector.tensor_copy(
                    atn01[:, cols], av01_t[fb][:, bcols]
                )
                nc.vector.tensor_copy(
                    atn2[:, cols],
                    av2_t[fb // 2][(fb % 2) * 64 : (fb % 2) * 64 + 64, bcols],
                )
                nc.sync.dma_start(ag_in[u][0:128, :], atn01[:, cols])
                nc.sync.dma_start(ag_in[u][128:EPG, :], atn2[:, cols])
                nc.gpsimd.collective_compute(
                    "AllGather",
                    mybir.AluOpType.bypass,
                    replica_groups=groups,
                    ins=[ag_in[u].ap().opt()],
                    outs=[ag_out[u].ap().opt()],
                )
                for dt_i in range(6):
                    nc.sync.dma_start(
                        ag_t[(fb, dt_i)][:, bcols],
                        ag_out[u][dt_i * 128 : (dt_i + 1) * 128, :],
                    )

            def outproj(f):
                cols = slice(512 * f, 512 * (f + 1))
                pt = arena.tile([128, 1024], f32, tag="strip")
                for dt_i in range(6):
                    nc.tensor.matmul(
                        pt[:, 0:512],
                        wout_t[dt_i][:, 0:128],
                        ag_t[(f, dt_i)][:, :],
                        start=(dt_i == 0), stop=(dt_i == 5),
                    )
                for dt_i in range(6):
                    nc.tensor.matmul(
                        pt[0:64, 512:1024],
                        wout_t[dt_i][:, 128:EPG],
                        ag_t[(f, dt_i)][:, :],
                        start=(dt_i == 0), stop=(dt_i == 5),
                    )
                o0 = o_p.tile([128, 512], f32, tag="o0")
                o1 = o_p.tile([64, 512], f32, tag="o1")
                nc.vector.tensor_scalar_add(o0[:], pt[:, 0:512], bout_c[:, 0:1])
                nc.vector.tensor_scalar_add(
                    o1[:], pt[0:64, 512:1024], bout_c[0:64, 1:2]
                )
                nc.sync.dma_start(out[0:128, cols], o0[:])
                nc.sync.dma_start(out[128:EPG, cols], o1[:])

            # ---- QKV projections up front ----
            qkv_chunk(qk2, 2, 0, 1)
            qkv_chunk(qk2, 2, 1, 1)
            qkv_chunk(q01, 1, 0, None)
            qkv_chunk(q01, 1, 1, None)
            qkv_chunk(k01, 0, 0, 0)
            qkv_chunk(k01, 0, 1, 0)
            nc.gpsimd.dma_start(k2[:], qk2[64:128, :])
            nc.gpsimd.dma_start(q2d[64:128, :], qk2[0:64, :])

            # ---- unified wave over ki; lazy AV past column 1024 ----
            # Software-pipelined by one step: strips for ki are emitted
            # before the normalizers/AV of ki-1 so the PE streams scores
            # while ScalarE exps the previous block.
            deferred = {}  # ki -> list of (hi, et, qp0, plen, vpt)

            def emit_strips(ki):
                q0 = 128 * ki
                parts = {hi: [] for hi in range(3)}
                bounds = [(q0, 1024), (1024, 2048)] if ki < 8 else [(q0, 2048)]
                for (p0, p1) in bounds:
                    for hi in range(3):
                        is_def = ki < 8 and p0 == 1024
                        et, acc = strip_part(ki, hi, p0, p1, is_def)
                        parts[hi].append((et, p0, p1, acc, is_def))
                return parts

            def emit_av(ki, parts):
                if 8 <= ki <= 11:
                    for old in (2 * (ki - 8), 2 * (ki - 8) + 1):
                        for (hi, et, qp0, plen, vpt) in deferred.pop(old, []):
                            av_apply(
                                old, hi, vpt, et, qp0, plen,
                                1024, qp0 + plen, first=(old == 0),
                            )
                for hi in range(3):
                    vpt = make_vpt([p[3] for p in parts[hi]], ki, hi)
                    for (et, p0, p1, _acc, is_def) in parts[hi]:
                        if is_def:
                            deferred.setdefault(ki, []).append(
                                (hi, et, p0, p1 - p0, vpt)
                            )
                        else:
                            hi_col = min(p1, 1024) if ki < 8 else p1
                            av_apply(
                                ki, hi, vpt, et, p0, p1 - p0,
                                p0, hi_col, first=(ki == 0),
                            )

            pipe = {}
            for it in range(18):
                if it < 16:
                    pipe[it] = emit_strips(it)
                if 1 <= it <= 8:
                    v_tile(2 * (it - 1))
                    v_tile(2 * (it - 1) + 1)
                if it == 11:
                    outproj(0)
                if it == 14:
                    outproj(1)
                av_ki = it - 2
                if av_ki >= 0:
                    emit_av(av_ki, pipe.pop(av_ki))
                    if av_ki == 3:
                        flush(0, 0, 512)
                    elif av_ki == 7:
                        flush(1, 512, 1024)
                    elif av_ki == 11:
                        flush(2, 1024, 1536)
                    elif av_ki == 13:
                        flush(3, 1536, 1792)
                    elif av_ki == 15:
                        flush(4, 1792, 2048)
            outproj(2)
            outproj(3)

    nc.compile()
    return nc


def _shards(x, W_in, b_in, W_out, b_out):
    """Build per-core input maps (host-side sharding / layout / bf16 prep)."""
    import ml_dtypes

    bf16 = ml_dtypes.bfloat16
    tri_np = np.where(
        np.arange(128)[None, :] < np.arange(128)[:, None], np.float32(NEG), 0.0
    ).astype(np.float32)
    zeros64 = np.zeros(64, dtype=np.float32)
    in_maps = []
    for c in range(NCORES):
        b = c // GROUPS
        g = c % GROUPS
        hs = [3 * g, 3 * g + 1, 3 * g + 2]
        qc = [W_in[:, 64 * h : 64 * (h + 1)] for h in hs]
        kc = [W_in[:, D + 64 * h : D + 64 * (h + 1)] for h in hs]
        vc = W_in[:, 2 * D + 64 * hs[0] : 2 * D + 64 * (hs[2] + 1)]
        kb = [b_in[D + 64 * h : D + 64 * (h + 1)] for h in hs]
        vb = b_in[2 * D + 64 * hs[0] : 2 * D + 64 * (hs[2] + 1)]
        # col order: [k0 k1 | q0 q1 | q2 k2]
        wqk = np.concatenate([kc[0], kc[1], qc[0], qc[1], qc[2], kc[2]], axis=1)
        bqkc = np.stack(
            [np.concatenate([kb[0], kb[1]]), np.concatenate([zeros64, kb[2]])],
            axis=1,
        ).astype(np.float32)
        bo = b_out[EPG * g : EPG * (g + 1)]
        boutc = np.stack(
            [bo[0:128], np.concatenate([bo[128:EPG], zeros64])], axis=1
        ).astype(np.float32)
        in_maps.append(
            {
                "xT": np.ascontiguousarray(x[b].T).astype(bf16),
                "wqk": np.ascontiguousarray(wqk).astype(bf16),
                "wv": np.ascontiguousarray(vc).astype(bf16),
                "wout": np.ascontiguousarray(
                    W_out[:, EPG * g : EPG * (g + 1)]
                ).astype(bf16),
                "bqkc": np.ascontiguousarray(bqkc),
                "bvv": np.ascontiguousarray(vb[None, :]).astype(bf16),
                "boutc": np.ascontiguousarray(boutc),
                "tri": tri_np,
            }
        )
    return in_maps


def _numpy_ref(x, mask, W_in, b_in, W_out, b_out):
    qkv = x @ W_in + b_in
    q, k, v = np.split(qkv, 3, axis=2)
    q = q.reshape(B, S, H, DH).transpose(0, 2, 1, 3)
    k = k.reshape(B, S, H, DH).transpose(0, 2, 1, 3)
    v = v.reshape(B, S, H, DH).transpose(0, 2, 1, 3)
    attn = np.einsum("bhqd,bhkd->bhqk", q, k) / np.sqrt(np.float32(D))
    attn = np.where(mask == 0, -np.inf, attn)
    attn = attn - attn.max(axis=-2, keepdims=True)
    e = np.exp(attn)
    attn = e / e.sum(axis=-2, keepdims=True)
    out = np.einsum("bhqk,bhkd->bhqd", attn, v)
    out = out.transpose(0, 2, 1, 3).reshape(B, S, D)
    return (out @ W_out + b_out).astype(np.float32)


def _run(inputs, trace=False):
    from concourse.bass_utils import run_bass_kernel_spmd

    x = np.asarray(inputs["x"], dtype=np.float32)
    mask = np.asarray(inputs["mask"])
    W_in = np.asarray(inputs["W_in"], dtype=np.float32)
    b_in = np.asarray(inputs["b_in"], dtype=np.float32)
    W_out = np.asarray(inputs["W_out"], dtype=np.float32)
    b_out = np.asarray(inputs["b_out"], dtype=np.float32)

    m2 = np.asarray(mask).reshape(S, S)
    if not np.array_equal(m2, np.tril(np.ones((S, S), m2.dtype))):
        return _numpy_ref(x, mask, W_in, b_in, W_out, b_out), None

    if "nc" not in _cache:
        _cache["nc"] = _build()
    nc = _cache["nc"]

    in_maps = _shards(x, W_in, b_in, W_out, b_out)
    res = run_bass_kernel_spmd(nc, in_maps, core_ids=list(range(NCORES)), trace=trace)

    full = np.empty((B, S, D), dtype=np.float32)
    for c in range(NCORES):
        b, g = c // GROUPS, c % GROUPS
        full[b, :, EPG * g : EPG * (g + 1)] = res.results[c]["out"].T
    return full, res


def kernel(**inputs) -> np.ndarray:
    out, _ = _run(inputs, trace=False)
    return out


# revision 19
# speedup vs baseline: 1.0518x; 1.0518x over previous
"""Trainium2 Bass kernel for nn_AttentionHead (B=2, S=2048, D=768, H=12).

Sharding: 8 cores = 2 batches x 4 head-groups (3 heads each).
Per core: QKV projection for its heads (transposed layout), causal
attention with softmax over the QUERY axis (reference peculiarity:
softmax dim=-2, scaled by sqrt(d_model)), AllGather of per-head outputs
within each batch's 4-core group, then a column-slice of the output
projection.  Host only slices / transposes / casts / concatenates.

v3 schedule notes:
  - All large inputs are pre-cast to bf16 on the host and DMA'd directly
    into matmul-ready SBUF tiles (no on-chip casts; half the HBM bytes).
  - Scores are built transposed (S_T[k, q]) so the softmax axis (q) is
    the SBUF free axis; ScalarE exp computes per-k row sums via
    accum_out.  The per-k normalizer is folded into V ("V'") so the
    attn @ v matmul consumes raw exp scores.
  - The q-projection bias is dropped: a per-k-row constant shifts all
    logits in the softmax (query) axis equally and cancels exactly.
  - One unified wave processes all three heads per ki block, keeping the
    PE and the (bottleneck) Scalar exp stream balanced end to end.
  - Lazy AV: for ki<8 the attn@v product is applied only to columns
    <1024; the exp tiles for columns >=1024 are retained in SBUF and
    applied at steps 8..11.  This keeps only 3 PSUM banks of AV state
    live, freeing 4 banks for two [128,1024] score arenas (fewer, wider
    exp instructions).
  - Four combined [192,512] AllGathers (one per 512-column block) carry
    all three heads; a dummy AllGather at t=0 absorbs the collective
    cold-start / barrier.  The output projection is interleaved into the
    wave as AllGather results land.
"""

import math

import numpy as np

B, S, D, H, DH = 2, 2048, 768, 12, 64
NCORES = 8
GROUPS = 4  # head-groups per batch
HPG = 3  # heads per group
EPG = HPG * DH  # 192
SCALE = 1.0 / math.sqrt(D)
NEG = -1.0e30

_cache = {}


def _build():
    import contextlib

    import concourse.bacc as bacc
    import concourse.mybir as mybir
    from concourse import tile

    f32 = mybir.dt.float32
    bf16 = mybir.dt.bfloat16
    EXP = mybir.ActivationFunctionType.Exp

    nc = bacc.Bacc("TRN2", target_bir_lowering=False, debug=False, num_devices=NCORES)

    xT = nc.dram_tensor("xT", [D, S], bf16, kind="ExternalInput")
    wqk = nc.dram_tensor("wqk", [D, 384], bf16, kind="ExternalInput")
    wv = nc.dram_tensor("wv", [D, EPG], bf16, kind="ExternalInput")
    wout = nc.dram_tensor("wout", [D, EPG], bf16, kind="ExternalInput")
    bqkc = nc.dram_tensor("bqkc", [128, 2], f32, kind="ExternalInput")
    bvv = nc.dram_tensor("bvv", [1, EPG], bf16, kind="ExternalInput")
    boutc = nc.dram_tensor("boutc", [128, 2], f32, kind="ExternalInput")
    tri = nc.dram_tensor("tri", [128, 128], f32, kind="ExternalInput")
    out = nc.dram_tensor("out", [EPG, S], f32, kind="ExternalOutput")

    ag_w = [512, 512, 512, 256, 256]
    ag_in = [
        nc.dram_tensor(f"ag_in{f}", [EPG, ag_w[f]], bf16) for f in range(5)
    ]
    ag_out = [
        nc.dram_tensor(f"ag_out{f}", [4 * EPG, ag_w[f]], bf16) for f in range(5)
    ]
    ag_ind = nc.dram_tensor("ag_ind", [64, 16], bf16)
    ag_outd = nc.dram_tensor("ag_outd", [256, 16], bf16)

    groups = [[0, 1, 2, 3], [4, 5, 6, 7]]

    with tile.TileContext(nc) as tc:
        with contextlib.ExitStack() as ctx:
            const_p = ctx.enter_context(tc.tile_pool(name="const", bufs=1))
            w_p = ctx.enter_context(tc.tile_pool(name="w", bufs=1))
            xt_p = ctx.enter_context(tc.tile_pool(name="xt", bufs=1))
            qk_p = ctx.enter_context(tc.tile_pool(name="qk", bufs=1))
            v_p = ctx.enter_context(tc.tile_pool(name="v", bufs=1))
            e_p = ctx.enter_context(tc.tile_pool(name="e", bufs=10))
            ed_p = ctx.enter_context(tc.tile_pool(name="ed", bufs=26))
            st_p = ctx.enter_context(tc.tile_pool(name="stat", bufs=24))
            vp_p = ctx.enter_context(tc.tile_pool(name="vp", bufs=30))
            atn_p = ctx.enter_context(tc.tile_pool(name="atn", bufs=1))
            ag_p = ctx.enter_context(tc.tile_pool(name="ag", bufs=1))
            o_p = ctx.enter_context(tc.tile_pool(name="o", bufs=2))
            arena = ctx.enter_context(tc.tile_pool(name="arena", bufs=2, space="PSUM"))
            avp = ctx.enter_context(tc.tile_pool(name="avp", bufs=1, space="PSUM"))

            # ---- constants (tiny DMAs first) ----
            bqk_c = const_p.tile([128, 2], f32)
            nc.sync.dma_start(bqk_c[:], bqkc[:, :])
            bout_c = const_p.tile([128, 2], f32)
            nc.sync.dma_start(bout_c[:], boutc[:, :])
            tri_f = const_p.tile([128, 128], f32)
            nc.sync.dma_start(tri_f[:], tri[:, :])
            bv_t = const_p.tile([1, EPG], bf16)
            nc.sync.dma_start(bv_t[:], bvv[:, :])
            ones = const_p.tile([1, 512], bf16)
            nc.vector.memset(ones[:], 1.0)
            warm = const_p.tile([128, 512], bf16)
            nc.vector.memset(warm[:], 0.0)
            dmy = const_p.tile([64, 16], bf16)
            nc.vector.memset(dmy[:], 0.0)

            # dummy AllGather: absorb the collective bootstrap barrier and
            # ncfw cold start while the PE is busy with QKV
            nc.sync.dma_start(ag_ind[:, :], dmy[:])
            nc.gpsimd.collective_compute(
                "AllGather",
                mybir.AluOpType.bypass,
                replica_groups=groups,
                ins=[ag_ind.ap().opt()],
                outs=[ag_outd.ap().opt()],
            )

            # ---- input DMAs: wqk first, then x halves, then wv, wout ----
            wqk_t = []
            for dt_i in range(6):
                wt = w_p.tile([128, 384], bf16, tag=f"wqk{dt_i}")
                nc.sync.dma_start(wt[:], wqk[dt_i * 128 : (dt_i + 1) * 128, :])
                wqk_t.append(wt)
            xt_t = []
            for dt_i in range(6):
                xt = xt_p.tile([128, S], bf16, tag=f"xt{dt_i}")
                nc.sync.dma_start(
                    xt[:, 0:1024], xT[dt_i * 128 : (dt_i + 1) * 128, 0:1024]
                )
                xt_t.append(xt)
            for dt_i in range(6):
                nc.sync.dma_start(
                    xt_t[dt_i][:, 1024:2048],
                    xT[dt_i * 128 : (dt_i + 1) * 128, 1024:2048],
                )
            wv_t, wout_t = [], []
            for dt_i in range(6):
                vt = w_p.tile([128, EPG], bf16, tag=f"wv{dt_i}")
                nc.sync.dma_start(vt[:], wv[dt_i * 128 : (dt_i + 1) * 128, :])
                wv_t.append(vt)
            for dt_i in range(6):
                wo = w_p.tile([128, EPG], bf16, tag=f"wout{dt_i}")
                nc.sync.dma_start(wo[:], wout[dt_i * 128 : (dt_i + 1) * 128, :])
                wout_t.append(wo)

            # ---- persistent SBUF tiles ----
            k01 = qk_p.tile([128, S], bf16, tag="k01")
            q01 = qk_p.tile([128, S], bf16, tag="q01")
            qk2 = qk_p.tile([128, S], bf16, tag="qk2")
            k2 = qk_p.tile([64, S], bf16, tag="k2")
            q2d = qk_p.tile([128, S], bf16, tag="q2d")
            vnat = v_p.tile([128, 16 * EPG], bf16)
            atn01 = atn_p.tile([128, S], bf16, tag="atn01")
            atn2 = atn_p.tile([64, S], bf16, tag="atn2")
            ag_t = {}
            for fb in range(4):
                for dt_i in range(6):
                    ag_t[(fb, dt_i)] = ag_p.tile(
                        [128, 512], bf16, tag=f"ag{fb}_{dt_i}", name=f"ag{fb}_{dt_i}"
                    )

            # ---- PE warm-up while first DMAs land ----
            for _ in range(20):
                wps = arena.tile([128, 1024], f32, tag="strip")
                nc.tensor.matmul(
                    wps[:, 0:512], warm[:, 0:128], warm[:],
                    start=True, stop=True, skip_group_check=True,
                )

            # qkv chunk: column group c (128 wide), seq chunk sc (1024 wide)
            def qkv_chunk(dst, c, sc, bias_col):
                pt = arena.tile([128, 1024], f32, tag="strip")
                for off in range(0, 1024, 512):
                    for dt_i in range(6):
                        nc.tensor.matmul(
                            pt[:, off : off + 512],
                            wqk_t[dt_i][:, c * 128 : (c + 1) * 128],
                            xt_t[dt_i][:, sc * 1024 + off : sc * 1024 + off + 512],
                            start=(dt_i == 0), stop=(dt_i == 5),
                        )
                if bias_col is None:
                    nc.vector.tensor_copy(dst[:, sc * 1024 : (sc + 1) * 1024], pt[:])
                else:
                    nc.vector.tensor_scalar_add(
                        dst[:, sc * 1024 : (sc + 1) * 1024],
                        pt[:],
                        bqk_c[:, bias_col : bias_col + 1],
                    )

            def v_tile(st_i):
                pt = arena.tile([128, 1024], f32, tag="strip")
                p = pt[:, 0:EPG]
                nc.tensor.matmul(p, ones[:, 0:128], bv_t[:], start=True, stop=False)
                for dt_i in range(6):
                    nc.tensor.matmul(
                        p,
                        xt_t[dt_i][:, st_i * 128 : (st_i + 1) * 128],
                        wv_t[dt_i][:],
                        start=False, stop=(dt_i == 5),
                    )
                nc.vector.tensor_copy(vnat[:, st_i * EPG : (st_i + 1) * EPG], p)

            # ---- AV state: 3 PSUM banks, block pairs recycled mid-wave ----
            av01_t = {}  # block -> [128,512] tile (h0 rows 0-63, h1 64-127)
            av2_t = {}  # pair index -> [128,512] tile (block 2t / 2t+1 halves)

            def av_slice(hi, b):
                if hi < 2:
                    if b not in av01_t:
                        av01_t[b] = avp.tile(
                            [128, 512], f32, tag="av01", name=f"av01_{b}", bufs=2
                        )
                    return av01_t[b][64 * hi : 64 * hi + 64, :]
                t = b // 2
                if t not in av2_t:
                    av2_t[t] = avp.tile([128, 512], f32, tag="av2", name=f"av2_{t}")
                return av2_t[t][(b % 2) * 64 : (b % 2) * 64 + 64, :]

            STOP_KI = {0: 3, 1: 7, 2: 11, 3: 15}

            def av_apply(ki, hi, vpt, et, qp0, plen, lo, hi_col, first):
                """AV chunks for global cols [lo, hi_col) of an et part."""
                q = lo
                while q < hi_col:
                    b = q // 512
                    qe = min((b + 1) * 512, hi_col)
                    dst = av_slice(hi, b)
                    nc.tensor.matmul(
                        dst[:, q - 512 * b : qe - 512 * b],
                        vpt[:],
                        et[:, q - qp0 : qe - qp0],
                        start=first, stop=(ki == STOP_KI[b]),
                        skip_group_check=True,
                    )
                    q = qe

            # score strip part: S_T[k, q] for q in [p0, p1); exp w/ accum
            def strip_part(ki, hi, p0, p1, deferred):
                kT, kb, qT, qb = srcs(ki, hi)
                plen = p1 - p0
                t = arena.tile([128, 1024], f32, tag="strip")
                off = 0
                while off < plen:
                    n = min(512, plen - off)
                    nc.tensor.matmul(
                        t[:, off : off + n],
                        kT[kb : kb + 64, ki * 128 : (ki + 1) * 128],
                        qT[qb : qb + 64, p0 + off : p0 + off + n],
                        start=True, stop=True, skip_group_check=True,
                    )
                    off += n
                if p0 == 128 * ki:
                    nc.vector.tensor_add(t[:, 0:128], t[:, 0:128], tri_f[:])
                et = (ed_p if deferred else e_p).tile(
                    [128, 1024], bf16, tag="ed" if deferred else "e"
                )
                acc = st_p.tile([128, 1], f32, tag="acc")
                nc.scalar.activation(
                    et[:, 0:plen], t[:, 0:plen], EXP, scale=SCALE, accum_out=acc[:]
                )
                return et, acc

            def srcs(ki, hi):
                if hi == 0:
                    return (k01, 0, q01, 0)
                if hi == 1:
                    return (k01, 64, q01, 64)
                return (k2, 0, qk2, 0) if ki % 2 == 0 else (qk2, 64, q2d, 64)

            def make_vpt(accs, ki, head):
                a = accs[0]
                for extra in accs[1:]:
                    s = st_p.tile([128, 1], f32, tag="ssum")
                    nc.vector.tensor_add(s[:], a[:], extra[:])
                    a = s
                rcp = st_p.tile([128, 1], f32, tag="rcp")
                nc.vector.reciprocal(rcp[:], a[:])
                vpt = vp_p.tile([128, 64], bf16, tag="vp")
                nc.vector.tensor_scalar_mul(
                    vpt[:],
                    vnat[:, ki * EPG + head * 64 : ki * EPG + (head + 1) * 64],
                    rcp[:],
                )
                return vpt

            def flush(u, c0, c1):
                w = c1 - c0
                fb = c0 // 512
                lo = c0 - 512 * fb
                cols = slice(c0, c1)
                bcols = slice(lo, lo + w)
                nc.vector.tensor_copy(
                    atn01[:, cols], av01_t[fb][:, bcols]
                )
                nc.vector.tensor_copy(
                    atn2[:, cols],
                    av2_t[fb // 2][(fb % 2) * 64 : (fb % 2) * 64 + 64, bcols],
                )
                nc.sync.dma_start(ag_in[u][0:128, :], atn01[:, cols])
                nc.sync.dma_start(ag_in[u][128:EPG, :], atn2[:, cols])
                nc.gpsimd.collective_compute(
                    "AllGather",
                    mybir.AluOpType.bypass,
                    replica_groups=groups,
                    ins=[ag_in[u].ap().opt()],
                    outs=[ag_out[u].ap().opt()],
                )
                for dt_i in range(6):
                    nc.sync.dma_start(
                        ag_t[(fb, dt_i)][:, bcols],
                        ag_out[u][dt_i * 128 : (dt_i + 1) * 128, :],
                    )

            def outproj(f, c0=None, c1=None):
                if c0 is None:
                    c0, c1 = 512 * f, 512 * (f + 1)
                w = c1 - c0
                lo = c0 - 512 * f
                cols = slice(c0, c1)
                pt = arena.tile([128, 1024], f32, tag="strip")
                for dt_i in range(6):
                    nc.tensor.matmul(
                        pt[:, 0:w],
                        wout_t[dt_i][:, 0:128],
                        ag_t[(f, dt_i)][:, lo : lo + w],
                        start=(dt_i == 0), stop=(dt_i == 5),
                    )
                for dt_i in range(6):
                    nc.tensor.matmul(
                        pt[0:64, 512 : 512 + w],
                        wout_t[dt_i][:, 128:EPG],
                        ag_t[(f, dt_i)][:, lo : lo + w],
                        start=(dt_i == 0), stop=(dt_i == 5),
                    )
                o0 = o_p.tile([128, 512], f32, tag="o0")
                o1 = o_p.tile([64, 512], f32, tag="o1")
                nc.vector.tensor_scalar_add(
                    o0[:, 0:w], pt[:, 0:w], bout_c[:, 0:1]
                )
                nc.vector.tensor_scalar_add(
                    o1[:, 0:w], pt[0:64, 512 : 512 + w], bout_c[0:64, 1:2]
                )
                nc.sync.dma_start(out[0:128, cols], o0[:, 0:w])
                nc.sync.dma_start(out[128:EPG, cols], o1[:, 0:w])

            # ---- QKV projections up front ----
            qkv_chunk(qk2, 2, 0, 1)
            qkv_chunk(q01, 1, 0, None)
            qkv_chunk(k01, 0, 0, 0)
            qkv_chunk(qk2, 2, 1, 1)
            qkv_chunk(q01, 1, 1, None)
            qkv_chunk(k01, 0, 1, 0)
            nc.gpsimd.dma_start(k2[:], qk2[64:128, :])
            nc.gpsimd.dma_start(q2d[64:128, :], qk2[0:64, :])

            # ---- unified wave over ki; lazy AV past column 1024 ----
            # Software-pipelined by one step: strips for ki are emitted
            # before the normalizers/AV of ki-1 so the PE streams scores
            # while ScalarE exps the previous block.
            deferred = {}  # ki -> list of (hi, et, qp0, plen, vpt)

            def emit_strips(ki):
                q0 = 128 * ki
                parts = {hi: [] for hi in range(3)}
                bounds = [(q0, 1024), (1024, 2048)] if ki < 8 else [(q0, 2048)]
                for (p0, p1) in bounds:
                    for hi in range(3):
                        is_def = ki < 8 and p0 == 1024
                        et, acc = strip_part(ki, hi, p0, p1, is_def)
                        parts[hi].append((et, p0, p1, acc, is_def))
                return parts

            def emit_deferred(ki):
                if 8 <= ki <= 11:
                    for old in (2 * (ki - 8), 2 * (ki - 8) + 1):
                        for (hi, et, qp0, plen, vpt) in deferred.pop(old, []):
                            av_apply(
                                old, hi, vpt, et, qp0, plen,
                                1024, qp0 + plen, first=(old == 0),
                            )

            def emit_av_head(ki, parts, hi):
                vpt = make_vpt([p[3] for p in parts[hi]], ki, hi)
                for (et, p0, p1, _acc, is_def) in parts[hi]:
                    if is_def:
                        deferred.setdefault(ki, []).append(
                            (hi, et, p0, p1 - p0, vpt)
                        )
                    else:
                        hi_col = min(p1, 1024) if ki < 8 else p1
                        av_apply(
                            ki, hi, vpt, et, p0, p1 - p0,
                            p0, hi_col, first=(ki == 0),
                        )

            prev = None
            for ki in range(16):
                parts = emit_strips(ki)
                if prev is not None:
                    av_ki = ki - 1
                    emit_deferred(av_ki)
                    emit_av_head(av_ki, prev, 0)
                    emit_av_head(av_ki, prev, 1)
                    if av_ki < 8:
                        v_tile(2 * av_ki)
                        v_tile(2 * av_ki + 1)
                    if ki == 10:
                        outproj(0)
                    if ki == 13:
                        outproj(1)
                    emit_av_head(av_ki, prev, 2)
                    if av_ki == 3:
                        flush(0, 0, 512)
                    elif av_ki == 7:
                        flush(1, 512, 1024)
                    elif av_ki == 11:
                        flush(2, 1024, 1536)
                    elif av_ki == 13:
                        flush(3, 1536, 1792)
                else:
                    v_tile(0)
                    v_tile(1)
                prev = parts
            emit_deferred(15)
            emit_av_head(15, prev, 0)
            emit_av_head(15, prev, 1)
            emit_av_head(15, prev, 2)
            flush(4, 1792, 2048)
            outproj(3, 1536, 1792)
            outproj(3, 1792, 2048)

    nc.compile()
    return nc


def _shards(x, W_in, b_in, W_out, b_out):
    """Build per-core input maps (host-side sharding / layout / bf16 prep)."""
    import ml_dtypes

    bf16 = ml_dtypes.bfloat16
    tri_np = np.where(
        np.arange(128)[None, :] < np.arange(128)[:, None], np.float32(NEG), 0.0
    ).astype(np.float32)
    zeros64 = np.zeros(64, dtype=np.float32)
    in_maps = []
    for c in range(NCORES):
        b = c // GROUPS
        g = c % GROUPS
        hs = [3 * g, 3 * g + 1, 3 * g + 2]
        qc = [W_in[:, 64 * h : 64 * (h + 1)] for h in hs]
        kc = [W_in[:, D + 64 * h : D + 64 * (h + 1)] for h in hs]
        vc = W_in[:, 2 * D + 64 * hs[0] : 2 * D + 64 * (hs[2] + 1)]
        kb = [b_in[D + 64 * h : D + 64 * (h + 1)] for h in hs]
        vb = b_in[2 * D + 64 * hs[0] : 2 * D + 64 * (hs[2] + 1)]
        # col order: [k0 k1 | q0 q1 | q2 k2]
        wqk = np.concatenate([kc[0], kc[1], qc[0], qc[1], qc[2], kc[2]], axis=1)
        bqkc = np.stack(
            [np.concatenate([kb[0], kb[1]]), np.concatenate([zeros64, kb[2]])],
            axis=1,
        ).astype(np.float32)
        bo = b_out[EPG * g : EPG * (g + 1)]
        boutc = np.stack(
            [bo[0:128], np.concatenate([bo[128:EPG], zeros64])], axis=1
        ).astype(np.float32)
        in_maps.append(
            {
                "xT": np.ascontiguousarray(x[b].T).astype(bf16),
                "wqk": np.ascontiguousarray(wqk).astype(bf16),
                "wv": np.ascontiguousarray(vc).astype(bf16),
                "wout": np.ascontiguousarray(
                    W_out[:, EPG * g : EPG * (g + 1)]
                ).astype(bf16),
                "bqkc": np.ascontiguousarray(bqkc),
                "bvv": np.ascontiguousarray(vb[None, :]).astype(bf16),
                "boutc": np.ascontiguousarray(boutc),
                "tri": tri_np,
            }
        )
    return in_maps


def _numpy_ref(x, mask, W_in, b_in, W_out, b_out):
    qkv = x @ W_in + b_in
    q, k, v = np.split(qkv, 3, axis=2)
    q = q.reshape(B, S, H, DH).transpose(0, 2, 1, 3)
    k = k.reshape(B, S, H, DH).transpose(0, 2, 1, 3)
    v = v.reshape(B, S, H, DH).transpose(0, 2, 1, 3)
    attn = np.einsum("bhqd,bhkd->bhqk", q, k) / np.sqrt(np.float32(D))
    attn = np.where(mask == 0, -np.inf, attn)
    attn = attn - attn.max(axis=-2, keepdims=True)
    e = np.exp(attn)
    attn = e / e.sum(axis=-2, keepdims=True)
    out = np.einsum("bhqk,bhkd->bhqd", attn, v)
    out = out.transpose(0, 2, 1, 3).reshape(B, S, D)
    return (out @ W_out + b_out).astype(np.float32)


def _run(inputs, trace=False):
    from concourse.bass_utils import run_bass_kernel_spmd

    x = np.asarray(inputs["x"], dtype=np.float32)
    mask = np.asarray(inputs["mask"])
    W_in = np.asarray(inputs["W_in"], dtype=np.float32)
    b_in = np.asarray(inputs["b_in"], dtype=np.float32)
    W_out = np.asarray(inputs["W_out"], dtype=np.float32)
    b_out = np.asarray(inputs["b_out"], dtype=np.float32)

    m2 = np.asarray(mask).reshape(S, S)
    if not np.array_equal(m2, np.tril(np.ones((S, S), m2.dtype))):
        return _numpy_ref(x, mask, W_in, b_in, W_out, b_out), None

    if "nc" not in _cache:
        _cache["nc"] = _build()
    nc = _cache["nc"]

    in_maps = _shards(x, W_in, b_in, W_out, b_out)
    res = run_bass_kernel_spmd(nc, in_maps, core_ids=list(range(NCORES)), trace=trace)

    full = np.empty((B, S, D), dtype=np.float32)
    for c in range(NCORES):
        b, g = c // GROUPS, c % GROUPS
        full[b, :, EPG * g : EPG * (g + 1)] = res.results[c]["out"].T
    return full, res


def kernel(**inputs) -> np.ndarray:
    out, _ = _run(inputs, trace=False)
    return out
